# revision 2
# baseline (speedup 1.0000x reference)
"""Trainium2 Bass kernel for nn_AggLineProbe (segment_reduce).

Full computation per reference:
  proj = emb @ W_down + b_down                  [T, 260]
  act  = [proj[:, :4], gelu(proj[:, 4:])]       (gelu = exact erf)
  token_logits   = act[:, 4:] @ W_final + b_final
  line/problem_logits = mha-pool(act, S) @ W_final + b_final  (softmax over span)
  line/problem_labels = labels.min over spans

Device strategy (8 cores, data-parallel over tokens; spans never cross shards):
  - per core 16384 tokens; 128 token-tiles of 128
  - SWDGE cast-DMA loads fp32 -> bf16 natural tiles (RNE, read-bound)
  - PE transposes (bf16, via identity) -> PSUM -> 4-wide ACT/DVE evacuation
    to build embT [d-on-partition] tiles
  - dense bf16 matmuls K=128x16 (+K=1 bias matmul) -> proj in PSUM [128, 260]
  - ScalarE gelu from PSUM; attention logits staged, exp batched per chunk
  - u[t,h] = gelu-values . W_final (per head) via DVE mul + segmented reduce
  - pooling sums via ones-matmul S [128, 9] (8 lines + problem partial),
    softmax normalization on DVE; labels min via DVE reduce_min
"""
import numpy as np
import ml_dtypes

NCORES = 8
T, D, H = 131072, 2048, 260
NV = H - 4
TS = T // NCORES           # 16384 tokens per core
P = 128
KG = D // P                # 16 k-groups
NTILE = TS // P            # 128 token-tiles per core
CHT = 16                   # tiles per chunk
NCH = NTILE // CHT         # 8 chunks
TPP, TPL = 512, 16
LPC = TS // TPL            # 1024 lines per core
PPC = TS // TPP            # 32 problems per core

_cache = {}


def _build(reps=1):
    from contextlib import ExitStack
    import concourse.mybir as mybir
    import concourse.tile as tile
    from concourse import bacc

    F32, BF16, I32 = mybir.dt.float32, mybir.dt.bfloat16, mybir.dt.int32
    AF = mybir.ActivationFunctionType
    MUL = mybir.AluOpType.mult
    X_AX = mybir.AxisListType.X

    nc = bacc.Bacc("TRN2", target_bir_lowering=False, debug=False)

    emb = nc.dram_tensor("emb", [TS, D], F32, kind="ExternalInput")
    labels = nc.dram_tensor("labels", [PPC, TPP], I32, kind="ExternalInput")
    w = nc.dram_tensor("w", [P, KG, H], BF16, kind="ExternalInput")
    bdown = nc.dram_tensor("bdown", [1, H], BF16, kind="ExternalInput")
    ones1 = nc.dram_tensor("ones1", [1, P], BF16, kind="ExternalInput")
    identb = nc.dram_tensor("identb", [P, P], BF16, kind="ExternalInput")
    identf = nc.dram_tensor("identf", [P, P], F32, kind="ExternalInput")
    wrep = nc.dram_tensor("wrep", [P, NV], F32, kind="ExternalInput")
    smat = nc.dram_tensor("smat", [P, 9], F32, kind="ExternalInput")
    bfrep = nc.dram_tensor("bfrep", [P, 1], F32, kind="ExternalInput")
    e9 = nc.dram_tensor("e9", [9, 1], F32, kind="ExternalInput")

    tl_out = nc.dram_tensor("tl_out", [TS], F32, kind="ExternalOutput")
    ll_out = nc.dram_tensor("ll_out", [LPC], F32, kind="ExternalOutput")
    pl_out = nc.dram_tensor("pl_out", [PPC], F32, kind="ExternalOutput")
    llab_out = nc.dram_tensor("llab_out", [LPC], I32, kind="ExternalOutput")
    plab_out = nc.dram_tensor("plab_out", [PPC], I32, kind="ExternalOutput")

    emb3 = emb.ap().rearrange("(n p) d -> n p d", p=P)     # [NTILE, 128, D]

    with tile.TileContext(nc) as tc, ExitStack() as ctx:
        cst = ctx.enter_context(tc.tile_pool(name="cst", bufs=1))
        natp = ctx.enter_context(tc.tile_pool(name="natp", bufs=4))
        embTp = ctx.enter_context(tc.tile_pool(name="embTp", bufs=4))
        gp = ctx.enter_context(tc.tile_pool(name="gp", bufs=3))
        stg = ctx.enter_context(tc.tile_pool(name="stg", bufs=2))
        outp = ctx.enter_context(tc.tile_pool(name="outp", bufs=2))
        evp = ctx.enter_context(tc.tile_pool(name="evp", bufs=2, space="PSUM"))
        projp = ctx.enter_context(tc.tile_pool(name="projp", bufs=3, space="PSUM"))
        smallp = ctx.enter_context(tc.tile_pool(name="smallp", bufs=2, space="PSUM"))
        pselp = ctx.enter_context(tc.tile_pool(name="pselp", bufs=1, space="PSUM"))

        wt = cst.tile([P, KG, H], BF16, tag="w"); nc.sync.dma_start(wt[:], w.ap())
        bd = cst.tile([1, H], BF16, tag="bd"); nc.sync.dma_start(bd[:], bdown.ap())
        on1 = cst.tile([1, P], BF16, tag="on1"); nc.sync.dma_start(on1[:], ones1.ap())
        idb = cst.tile([P, P], BF16, tag="idb"); nc.sync.dma_start(idb[:], identb.ap())
        idf = cst.tile([P, P], F32, tag="idf"); nc.sync.dma_start(idf[:], identf.ap())
        wr = cst.tile([P, NV], F32, tag="wr"); nc.sync.dma_start(wr[:], wrep.ap())
        sm = cst.tile([P, 9], F32, tag="sm"); nc.sync.dma_start(sm[:], smat.ap())
        bfr = cst.tile([P, 1], F32, tag="bfr"); nc.sync.dma_start(bfr[:], bfrep.ap())

        e9t = cst.tile([9, 1], F32, tag="e9"); nc.sync.dma_start(e9t[:], e9.ap())
        pools_all = cst.tile([9, NTILE, 8], F32, tag="poolsall")

        def body():
            for c in range(NCH):
                attn_st = stg.tile([P, CHT, 4], F32, tag="attn_st")
                u_st = stg.tile([P, CHT, 4], F32, tag="u_st")
                X = stg.tile([P, CHT, 8], F32, tag="X")
                for i in range(CHT):
                    ti = c * CHT + i
                    nat = natp.tile([P, D], BF16, tag="nat")
                    nc.gpsimd.dma_start(nat[:], emb3[ti])
                    embT = embTp.tile([P, KG, P], BF16, tag="embT")
                    for jj in range(0, KG, 4):
                        ev = evp.tile([P, 4, P], BF16, tag="ev")
                        for j in range(jj, jj + 4):
                            nc.tensor.transpose(
                                ev[:, j - jj, :], nat[:, j * P : (j + 1) * P], idb[:]
                            )
                        if (jj // 4) % 2 == 0:
                            nc.vector.tensor_copy(embT[:, jj : jj + 4, :], ev[:])
                        else:
                            nc.scalar.copy(embT[:, jj : jj + 4, :], ev[:])
                    proj = projp.tile([P, H], F32, tag="proj")
                    nc.tensor.matmul(proj[:], on1[:], bd[:], start=True, stop=False)
                    for k in range(KG):
                        nc.tensor.matmul(
                            proj[:], embT[:, k, :], wt[:, k, :],
                            start=False, stop=(k == KG - 1),
                        )
                    g = gp.tile([P, NV], F32, tag="g")
                    nc.scalar.activation(g[:], proj[:, 4:H], AF.Gelu)
                    nc.vector.tensor_copy(attn_st[:, i, :], proj[:, 0:4])
                    gu = gp.tile([P, NV], F32, tag="gu")
                    nc.vector.tensor_tensor(gu[:], g[:], wr[:], MUL)
                    nc.vector.reduce_sum(
                        u_st[:, i, :],
                        gu[:].rearrange("p (h d) -> p h d", h=4),
                        axis=X_AX,
                    )
                # chunk tail
                nc.scalar.activation(X[:, :, 0:4], attn_st[:], AF.Exp)
                nc.vector.tensor_tensor(X[:, :, 4:8], X[:, :, 0:4], u_st[:], MUL)
                tl = outp.tile([P, CHT], F32, tag="tl")
                nc.vector.reduce_sum(tl[:], u_st[:], axis=X_AX)
                nc.scalar.activation(tl[:], tl[:], AF.Identity, bias=bfr[:])
                tlT = smallp.tile([CHT, P], F32, tag="sm8")
                nc.tensor.transpose(tlT[:], tl[:], idf[:])
                tlTs = outp.tile([CHT, P], F32, tag="tlTs")
                nc.vector.tensor_copy(tlTs[:], tlT[:])
                nc.gpsimd.dma_start(
                    tl_out.ap()[c * CHT * P : (c + 1) * CHT * P].rearrange(
                        "(i p) -> i p", p=P
                    ),
                    tlTs[:],
                )
                for i in range(CHT):
                    pp = smallp.tile([CHT, P], F32, tag="sm8")
                    nc.tensor.matmul(
                        pp[0:9, 0:8], sm[:], X[:, i, :], start=True, stop=True
                    )
                    nc.vector.tensor_copy(pools_all[:, c * CHT + i, :], pp[0:9, 0:8])
                pools = pools_all[:, c * CHT : (c + 1) * CHT, :]
                # line logits for this chunk (128 lines)
                rec = outp.tile([8, CHT, 4], F32, tag="rec")
                nc.vector.reciprocal(rec[:], pools[0:8, :, 0:4])
                nc.vector.tensor_tensor(rec[:], rec[:], pools[0:8, :, 4:8], MUL)
                ll = outp.tile([8, CHT], F32, tag="ll")
                nc.vector.reduce_sum(ll[:], rec[:], axis=X_AX)
                nc.scalar.activation(ll[:], ll[:], AF.Identity, bias=bfr[0:8, :])
                llT = smallp.tile([CHT, P], F32, tag="sm8")
                nc.tensor.transpose(llT[0:CHT, 0:8], ll[:], idf[0:8, 0:8])
                llTs = outp.tile([CHT, 8], F32, tag="llTs")
                nc.vector.tensor_copy(llTs[:], llT[0:CHT, 0:8])
                nc.gpsimd.dma_start(
                    ll_out.ap()[c * CHT * 8 : (c + 1) * CHT * 8].rearrange(
                        "(i q) -> i q", q=8
                    ),
                    llTs[:],
                )

        if reps == 1:
            body()
        else:
            with tc.For_i(0, reps):
                body()

        # problem logits (once): select partial row 8 via e9 MM, group-sum, normalize
        pools_flat = pools_all[:].rearrange("p t v -> p (t v)")
        pprt = outp.tile([1, NTILE * 8], F32, tag="pprt")
        for j in range(2):
            psel = pselp.tile([1, 512], F32, tag="psel")
            nc.tensor.matmul(psel[:], e9t[:], pools_flat[:, j * 512 : (j + 1) * 512],
                             start=True, stop=True)
            nc.vector.tensor_copy(pprt[:, j * 512 : (j + 1) * 512], psel[:])
        pacc2 = outp.tile([1, PPC, 8], F32, tag="pacc2")
        nc.vector.reduce_sum(
            pacc2[:],
            pprt[:].rearrange("o (grp four v) -> o grp v four", four=4, v=8),
            axis=X_AX,
        )
        prec = outp.tile([1, PPC, 4], F32, tag="prec")
        nc.vector.reciprocal(prec[:], pacc2[:, :, 0:4])
        nc.vector.tensor_tensor(prec[:], prec[:], pacc2[:, :, 4:8], MUL)
        pl = outp.tile([1, PPC], F32, tag="pl")
        nc.vector.reduce_sum(pl[:], prec[:], axis=X_AX)
        nc.scalar.activation(pl[:], pl[:], AF.Identity, bias=bfr[0:1, :])
        nc.gpsimd.dma_start(pl_out.ap().rearrange("(o q) -> o q", o=1), pl[:])

        # labels
        lab = outp.tile([PPC, TPP], I32, tag="lab")
        nc.gpsimd.dma_start(lab[:], labels.ap())
        labf = outp.tile([PPC, TPP], F32, tag="labf")
        nc.vector.tensor_copy(labf[:], lab[:])
        lmin = outp.tile([PPC, TPP // TPL], F32, tag="lmin")
        nc.vector.tensor_reduce(
            lmin[:], labf[:].rearrange("p (l s) -> p l s", s=TPL),
            axis=X_AX, op=mybir.AluOpType.min,
        )
        lmini = outp.tile([PPC, TPP // TPL], I32, tag="lmini")
        nc.vector.tensor_copy(lmini[:], lmin[:])
        nc.gpsimd.dma_start(
            llab_out.ap().rearrange("(p l) -> p l", p=PPC), lmini[:]
        )
        pmin = outp.tile([PPC, 1], F32, tag="pmin")
        nc.vector.tensor_reduce(pmin[:], labf[:], axis=X_AX, op=mybir.AluOpType.min)
        pmini = outp.tile([PPC, 1], I32, tag="pmini")
        nc.vector.tensor_copy(pmini[:], pmin[:])
        nc.gpsimd.dma_start(plab_out.ap().rearrange("(p o) -> p o", o=1), pmini[:])

    nc.compile()
    return nc


def _get_nc(reps=1):
    if reps not in _cache:
        _cache[reps] = _build(reps)
    return _cache[reps]


def _make_consts(W_down, b_down, W_final, b_final):
    Wb = np.asarray(W_down, np.float32).astype(ml_dtypes.bfloat16)
    consts = {
        "w": np.ascontiguousarray(Wb.reshape(KG, P, H).transpose(1, 0, 2)),
        "bdown": np.asarray(b_down, np.float32).astype(ml_dtypes.bfloat16)[None, :],
        "ones1": np.ones((1, P), ml_dtypes.bfloat16),
        "identb": np.eye(P, dtype=ml_dtypes.bfloat16),
        "identf": np.eye(P, dtype=np.float32),
        "wrep": np.tile(np.asarray(W_final, np.float32)[:, 0][None, :], (P, 1)),
        "smat": np.concatenate(
            [
                (np.arange(P)[:, None] // TPL == np.arange(8)[None, :]).astype(np.float32),
                np.ones((P, 1), np.float32),
            ],
            axis=1,
        ),
        "bfrep": np.full((P, 1), np.float32(np.asarray(b_final).reshape(-1)[0])),
        "e9": np.concatenate([np.zeros((8, 1), np.float32), np.ones((1, 1), np.float32)]),
    }
    return consts


def run(embeddings, W_down, b_down, W_final, b_final, labels, reps=1):
    from concourse.bass_utils import run_bass_kernel_spmd

    nc = _get_nc(reps)
    consts = _make_consts(W_down, b_down, W_final, b_final)
    embeddings = np.asarray(embeddings, np.float32)
    labels_np = np.asarray(labels, np.int32)
    in_maps = []
    for c in range(NCORES):
        in_maps.append(
            {
                "emb": embeddings[c * TS : (c + 1) * TS],
                "labels": labels_np[c * TS : (c + 1) * TS].reshape(PPC, TPP),
                **consts,
            }
        )
    res = run_bass_kernel_spmd(nc, in_maps, core_ids=list(range(NCORES)))
    rs = res.results
    token_logits = np.concatenate([r["tl_out"] for r in rs])
    line_logits = np.concatenate([r["ll_out"] for r in rs])
    problem_logits = np.concatenate([r["pl_out"] for r in rs])
    line_labels = np.concatenate([r["llab_out"] for r in rs])
    problem_labels = np.concatenate([r["plab_out"] for r in rs])
    return token_logits, line_logits, line_labels, problem_logits, problem_labels


def kernel(embeddings, W_down, b_down, W_final, b_final, labels,
           tokens_per_problem, tokens_per_line):
    assert int(tokens_per_problem) == TPP and int(tokens_per_line) == TPL
    return run(embeddings, W_down, b_down, W_final, b_final, labels)
